# revision 1
# baseline (speedup 1.0000x reference)
"""Trainium2 Bass kernel for the DH-LIF node single-step forward.

Math: the mask is one-hot over the branch dim NB, so

    spike = ( (1-beta) * (x @ (W + 0.5*M_eff).T + b) >= 1 )
    M_eff[h,i] = oma[h, idx[h,i]],   oma[h,k] = 0.5*(1 - sigmoid(tau_n[h,k]))

where idx[h,i] is the branch the (dense, one-hot) mask assigns to input i of
hidden unit h.  The host losslessly re-encodes the one-hot mask as that index
plane (fp8, values 0..3); the device reconstructs M_eff with a per-partition
cubic through the 4 oma values (exact at the integer nodes), builds
Wc = W + M_eff, transposes it to i-major on TensorE, splits hi/lo bf16 (keeps
fp32-level precision at bf16 matmul speed), and accumulates
out[h, b] = Wc_T.T @ x_T over 32 k-chunks.  Threshold compares against the
per-partition value 1/(1-beta) - b.

Sharding: hidden dim split across 8 cores (h_loc = 256); x replicated.
Host does layout/dtype prep (transpose of x, index extraction, sharding) and
the final gather/transpose.
"""

import numpy as np
import ml_dtypes

B, I, H, NB = 512, 4096, 2048, 4
NCORES = 8
H_LOC = H // NCORES          # 256
N_HT = H_LOC // 128          # 2 partition tiles of hidden per core
S = 512                      # i-supertile size for the Wc build
N_SUPER = I // S             # supers per h-tile
G = 512                      # transpose/psum group width (4x 128x128 tiles)
N_GROUPS = S // G            # groups per supertile
N_CHUNK = I // 128           # 32 matmul k-chunks

TRACE = False
LAST_RESULTS = None
_CACHED = {}


def _build_bass(reps=1):
    import concourse.bacc as bacc
    import concourse.mybir as mybir
    from concourse.tile import TileContext
    from concourse.masks import make_identity

    f32 = mybir.dt.float32
    bf16 = mybir.dt.bfloat16
    fp8 = mybir.dt.float8e4
    AF = mybir.ActivationFunctionType
    ALU = mybir.AluOpType

    nc = bacc.Bacc("TRN2", target_bir_lowering=False, debug=False)

    xT = nc.dram_tensor("xT", [I, B], bf16, kind="ExternalInput")
    w_in = nc.dram_tensor("w", [H_LOC, I], f32, kind="ExternalInput")
    idx_in = nc.dram_tensor("idx", [H_LOC, I], fp8, kind="ExternalInput")
    tau_n = nc.dram_tensor("tau_n", [H_LOC, NB], f32, kind="ExternalInput")
    tau_m = nc.dram_tensor("tau_m", [H_LOC, 1], f32, kind="ExternalInput")
    b_in = nc.dram_tensor("b", [H_LOC, 1], f32, kind="ExternalInput")
    out = nc.dram_tensor("out", [H_LOC, B], f32, kind="ExternalOutput")

    # x viewed as [chunk-groups, 128, 4, 512] for SBUF tiles
    xT_v = xT.rearrange("(g j p) b -> g p j b", p=128, j=4)
    n_xg = xT_v.shape[0]  # 8

    with TileContext(nc) as tc:
        with (
            tc.tile_pool(name="const", bufs=1) as const_pool,
            tc.tile_pool(name="xp", bufs=n_xg) as x_pool,
            tc.tile_pool(name="ix", bufs=N_HT * N_SUPER) as idx_pool,
            tc.tile_pool(name="wp", bufs=N_HT * N_SUPER) as w_pool,
            tc.tile_pool(name="hb", bufs=4) as h_pool,
            tc.tile_pool(name="ub", bufs=4) as u_pool,
            tc.tile_pool(name="hi", bufs=4) as hi_pool,
            tc.tile_pool(name="lo", bufs=4) as lo_pool,
            tc.tile_pool(name="res", bufs=2) as res_pool,
            tc.tile_pool(name="pt", bufs=3, space="PSUM") as psum_t_pool,
            tc.tile_pool(name="po", bufs=2, space="PSUM") as psum_o_pool,
            tc.tile_pool(name="pw", bufs=1, space="PSUM") as psum_w_pool,
        ):
            ident = const_pool.tile([128, 128], f32)
            make_identity(nc, ident)

            # HAM warmup: the PE sits idle for the first ~10us while Wc is
            # built, and its clock gate (PE_HAM) would hold it at 1.2 GHz for
            # the first ~3.4us of real matmuls.  Fill the idle window with
            # dummy matmuls so the array enters the kernel warm (2.4 GHz).
            warm = psum_w_pool.tile([128, 128], f32, name="warm")
            for wi in range(20):
                nc.tensor.matmul(warm[:], ident[:], ident[:],
                                 start=True, stop=True, skip_group_check=True)
            for rep in range(reps):
                _emit_rep(nc, tc, rep, ident,
                          const_pool, x_pool, idx_pool, w_pool, h_pool,
                          u_pool, hi_pool, lo_pool, res_pool,
                          psum_t_pool, psum_o_pool,
                          xT_v, n_xg, w_in, idx_in, tau_n, tau_m, b_in, out,
                          f32, bf16, fp8, AF, ALU)

    nc.compile()
    return nc


def _emit_rep(nc, tc, rep, ident,
              const_pool, x_pool, idx_pool, w_pool, h_pool,
              u_pool, hi_pool, lo_pool, res_pool,
              psum_t_pool, psum_o_pool,
              xT_v, n_xg, w_in, idx_in, tau_n, tau_m, b_in, out,
              f32, bf16, fp8, AF, ALU):
    R = f"r{rep}_"

    # tiny parameter DMAs first so they land ahead of the bulk traffic in
    # the DMA queues — the whole Wc build depends on them
    param_tiles = []
    for ht in range(N_HT):
        hs = slice(ht * 128, (ht + 1) * 128)
        tn = const_pool.tile([128, NB], f32, tag=f"{R}tn{ht}", name=f"{R}tn{ht}")
        tm = const_pool.tile([128, 1], f32, tag=f"{R}tm{ht}", name=f"{R}tm{ht}")
        bv = const_pool.tile([128, 1], f32, tag=f"{R}bv{ht}", name=f"{R}bv{ht}")
        nc.sync.dma_start(tn[:], tau_n[hs, :])
        nc.sync.dma_start(tm[:], tau_m[hs, :])
        nc.sync.dma_start(bv[:], b_in[hs, :])
        param_tiles.append((tn, tm, bv))

    # Pre-allocate all idx/W tiles and emit their DMAs interleaved with the
    # x tiles, first supertile first, so the Wc build can start immediately
    # while x streams in behind it.
    x_sb = [None] * n_xg
    idx_sb = {}
    w_sb = {}
    xg_next = [0]

    def dma_x(n):
        for _ in range(n):
            if xg_next[0] < n_xg:
                g = xg_next[0]
                xt = x_pool.tile([128, 4, B], bf16, tag="xsb", name=f"{R}x{g}")
                nc.sync.dma_start(xt[:], xT_v[g])
                x_sb[g] = xt
                xg_next[0] += 1

    for ht in range(N_HT):
        hs = slice(ht * 128, (ht + 1) * 128)
        for ig in range(N_SUPER):
            isl = slice(ig * S, (ig + 1) * S)
            it = idx_pool.tile([128, S], fp8, tag="ix", name=f"{R}ix{ht}_{ig}")
            nc.sync.dma_start(it[:], idx_in[hs, isl])
            wt = w_pool.tile([128, S], f32, tag="wp", name=f"{R}w{ht}_{ig}")
            nc.sync.dma_start(wt[:], w_in[hs, isl])
            idx_sb[(ht, ig)] = it
            w_sb[(ht, ig)] = wt
            dma_x(1)
    dma_x(n_xg)

    # per-h-tile parameters: polynomial coefs for M_eff and threshold
    coef = []   # (a_ap, b_ap, c_ap, d_ap) per ht
    thr_t = []
    for ht in range(N_HT):
        tn, tm, bv = param_tiles[ht]
        sig_n = const_pool.tile([128, NB], f32, tag=f"{R}sn{ht}", name=f"{R}sn{ht}")
        nc.scalar.activation(sig_n[:], tn[:], AF.Sigmoid)
        oma = const_pool.tile([128, NB], f32, tag=f"{R}oma{ht}", name=f"{R}oma{ht}")
        # 0.5 * (1 - sigmoid(tau_n)) — includes the 0.5 dendritic scale
        nc.vector.tensor_scalar(oma[:], sig_n[:], -0.5, 0.5, op0=ALU.mult, op1=ALU.add)

        # Newton -> monomial coefficients of the cubic through (k, oma[:,k]),
        # k=0..3:  p(x) = a + b x + c x^2 + d x^3
        sc = const_pool.tile([128, 12], f32, tag=f"{R}sc{ht}", name=f"{R}sc{ht}")
        o = lambda k: oma[:, k : k + 1]
        d3 = sc[:, 0:3]                       # first differences
        dd2 = sc[:, 3:5]                      # second differences
        ddd = sc[:, 5:6]                      # third difference
        b_c, c_c, d_c = sc[:, 6:7], sc[:, 7:8], sc[:, 8:9]
        t1, t2 = sc[:, 9:10], sc[:, 10:11]
        nc.vector.tensor_tensor(d3, oma[:, 1:4], oma[:, 0:3], ALU.subtract)
        nc.vector.tensor_tensor(dd2, d3[:, 1:3], d3[:, 0:2], ALU.subtract)
        nc.vector.tensor_tensor(ddd, dd2[:, 1:2], dd2[:, 0:1], ALU.subtract)
        d0, dd0 = d3[:, 0:1], dd2[:, 0:1]
        # d = ddd/6, c = (dd0 - ddd)/2 first (they gate the Horner start)
        nc.scalar.mul(d_c, ddd, 1.0 / 6.0)
        nc.vector.tensor_tensor(c_c, dd0, ddd, ALU.subtract)
        nc.vector.tensor_scalar(c_c, c_c, 0.5, None, op0=ALU.mult)
        # b = d0 - dd0/2 + ddd/3
        nc.scalar.mul(t2, ddd, 1.0 / 3.0)
        nc.vector.tensor_scalar(t1, dd0, -0.5, None, op0=ALU.mult)
        nc.vector.tensor_tensor(t1, t1, d0, ALU.add)
        nc.vector.tensor_tensor(b_c, t1, t2, ALU.add)
        coef.append((o(0), b_c, c_c, d_c))

        beta = const_pool.tile([128, 1], f32, tag=f"{R}be{ht}", name=f"{R}be{ht}")
        nc.scalar.activation(beta[:], tm[:], AF.Sigmoid)
        omb = const_pool.tile([128, 1], f32, tag=f"{R}ob{ht}", name=f"{R}ob{ht}")
        nc.vector.tensor_scalar(omb[:], beta[:], -1.0, 1.0, op0=ALU.mult, op1=ALU.add)
        rb = const_pool.tile([128, 1], f32, tag=f"{R}rb{ht}", name=f"{R}rb{ht}")
        nc.vector.reciprocal(rb[:], omb[:])
        thr = const_pool.tile([128, 1], f32, tag=f"{R}th{ht}", name=f"{R}th{ht}")
        nc.vector.tensor_tensor(thr[:], rb[:], bv[:], ALU.subtract)
        thr_t.append(thr)

    # main pipeline
    SKEW = 2
    pending = []  # [(hi_tile, lo_tile, base_chunk, ht)]
    psum_out = [None] * N_HT

    def issue_mms(pend):
        hi_t, lo_t, base_c, ht_ = pend
        po = psum_out[ht_]
        is_last_group = base_c == (N_CHUNK - 4)
        for j in range(4):
            c = base_c + j
            xg, xj = divmod(c, 4)
            nc.tensor.matmul(
                po[:], hi_t[:, j * 128 : (j + 1) * 128],
                x_sb[xg][:, xj, :],
                start=(c == 0), stop=False, skip_group_check=True,
            )
            nc.tensor.matmul(
                po[:], lo_t[:, j * 128 : (j + 1) * 128],
                x_sb[xg][:, xj, :],
                start=False, stop=(is_last_group and j == 3),
                skip_group_check=True,
            )

    # Software-pipelined emission over all (ht, ig) supers, in the same order
    # as their DMAs were issued.
    supers = [(ht, ig) for ht in range(N_HT) for ig in range(N_SUPER)]
    n_sup = len(supers)
    P_t = [None] * n_sup

    for ht in range(N_HT):
        psum_out[ht] = psum_o_pool.tile([128, B], f32, tag="po", name=f"{R}po{ht}")

    def s0_ts1(k):  # DVE: P = d*idx + c  (2x mode; Pool keeps only TT4)
        ht, ig = supers[k]
        _, _, c_ap, d_ap = coef[ht]
        P = h_pool.tile([128, S], f32, tag="hb", name=f"{R}P{k}")
        P_t[k] = P
        nc.vector.tensor_scalar(P[:], idx_sb[(ht, ig)][:], d_ap, c_ap,
                                op0=ALU.mult, op1=ALU.add)

    def s1_tt1(k):  # DVE: P *= idx
        ht, ig = supers[k]
        nc.vector.tensor_tensor(P_t[k][:], P_t[k][:], idx_sb[(ht, ig)][:], ALU.mult)

    def s2_addb(k):  # Act: P += b
        ht, ig = supers[k]
        nc.scalar.add(P_t[k][:], P_t[k][:], coef[ht][1])

    def s3_tt2(k):  # DVE: P *= idx
        ht, ig = supers[k]
        nc.vector.tensor_tensor(P_t[k][:], P_t[k][:], idx_sb[(ht, ig)][:], ALU.mult)

    def s4_addw(k):  # Pool: P += W
        ht, ig = supers[k]
        nc.gpsimd.tensor_tensor(P_t[k][:], P_t[k][:], w_sb[(ht, ig)][:], ALU.add)

    def s5_consume(k):  # Act +a into wc + PE transposes + hi/lo + MMs
        ht, ig = supers[k]
        a_ap = coef[ht][0]
        wctile = u_pool.tile([128, S], f32, tag="ub", name=f"{R}wc{k}")
        for g in range(N_GROUPS):
            gsl = slice(g * G, (g + 1) * G)
            nc.scalar.add(wctile[:, gsl], P_t[k][:, gsl], a_ap)
            pt = psum_t_pool.tile([128, G], f32, tag="pt", name=f"{R}pt{k}_{g}")
            for j in range(4):
                col = g * G + j * 128
                nc.tensor.transpose(
                    pt[:, j * 128 : (j + 1) * 128],
                    wctile[:, col : col + 128],
                    ident[:],
                )
            hi_t = hi_pool.tile([128, G], bf16, tag="hi", name=f"{R}hi{k}_{g}")
            lo_t = lo_pool.tile([128, G], bf16, tag="lo", name=f"{R}lo{k}_{g}")
            nc.scalar.copy(hi_t[:], pt[:])
            nc.vector.tensor_tensor(lo_t[:], pt[:], hi_t[:], ALU.subtract)
            pending.append((hi_t, lo_t, (ig * N_GROUPS + g) * 4, ht))
            if len(pending) > SKEW:
                issue_mms(pending.pop(0))

    # prologue
    s0_ts1(0)
    s0_ts1(1)
    s1_tt1(0)
    for k in range(n_sup):
        s2_addb(k)
        if k + 1 < n_sup:
            s1_tt1(k + 1)
        s3_tt2(k)
        if k + 2 < n_sup:
            s0_ts1(k + 2)
        s4_addw(k)
        s5_consume(k)

    for pend in pending:
        issue_mms(pend)

    for ht in range(N_HT):
        res = res_pool.tile([128, B], f32, tag="res", name=f"{R}res{ht}")
        nc.vector.tensor_scalar(
            res[:], psum_out[ht][:], thr_t[ht][:], None, op0=ALU.is_ge
        )
        nc.sync.dma_start(out[ht * 128 : (ht + 1) * 128, :], res[:])


def _get_nc(reps=1):
    key = f"nc{reps}"
    if key not in _CACHED:
        _CACHED[key] = _build_bass(reps)
    return _CACHED[key]


def kernel(**inputs):
    global LAST_RESULTS
    from concourse.bass_utils import run_bass_kernel_spmd

    x = np.asarray(inputs["x"], dtype=np.float32)
    W = np.asarray(inputs["W"], dtype=np.float32)
    b = np.asarray(inputs["b"], dtype=np.float32)
    tau_m = np.asarray(inputs["tau_m"], dtype=np.float32)
    tau_n = np.asarray(inputs["tau_n"], dtype=np.float32)
    mask = np.asarray(inputs["mask"], dtype=np.float32)

    bf16 = ml_dtypes.bfloat16
    fp8 = ml_dtypes.float8_e4m3
    xT = np.ascontiguousarray(x.T).astype(bf16)                      # [I, B]
    # branch index of each (h, i): mask is one-hot over k (exact 0/1 values)
    idx = (
        mask[:, :, 1] + 2.0 * mask[:, :, 2] + 3.0 * mask[:, :, 3]
    ).astype(fp8)                                                     # [H, I]

    nc = _get_nc()
    in_maps = []
    for c in range(NCORES):
        hs = slice(c * H_LOC, (c + 1) * H_LOC)
        in_maps.append({
            "xT": xT,
            "w": np.ascontiguousarray(W[hs]),
            "idx": np.ascontiguousarray(idx[hs]),
            "tau_n": np.ascontiguousarray(tau_n[hs]),
            "tau_m": np.ascontiguousarray(tau_m[hs, None]),
            "b": np.ascontiguousarray(b[hs, None]),
        })

    try:
        res = run_bass_kernel_spmd(
            nc, in_maps, core_ids=list(range(NCORES)), trace=TRACE,
        )
    except Exception:
        if not TRACE:
            raise
        # tracing needs the NTFF profiling hook, which not every
        # environment provides — rerun without it
        res = run_bass_kernel_spmd(
            nc, in_maps, core_ids=list(range(NCORES)), trace=False,
        )
    LAST_RESULTS = res
    outT = np.concatenate([r["out"] for r in res.results], axis=0)   # [H, B]
    return np.ascontiguousarray(outT.T)                               # [B, H]



# revision 2
# speedup vs baseline: 1.6467x; 1.6467x over previous
"""Trainium2 Bass kernel for the DH-LIF node single-step forward.

Math: the mask is one-hot over the branch dim NB, so

    spike = ( (1-beta) * (x @ (W + M).T + b) >= 1 )
    M[h,i] = oma[h, idx[h,i]],   oma[h,k] = 0.5*(1 - sigmoid(tau_n[h,k]))

where idx[h,i] is the branch the one-hot mask assigns to input i of hidden
unit h.  The host losslessly re-encodes the mask as that index plane (fp16,
values 0..3); the device reconstructs M with a per-partition cubic through
the 4 oma values (exact at the integer nodes, Horner in fp16), transposes
the fp16 M plane on TensorE, fuses the +W^T into the PSUM->SBUF copy
(W^T ships pre-transposed fp16 from the host), and runs a single fp16 x fp8
matmul per k-chunk (x ships as fp8; spikes are 0/1, exact).  Threshold
compares the f32 PSUM against 1/(1-beta) - b per partition; the 0/1 result
is written back as fp8.

Engine split per 1024-wide i-supertile (all 16-bit to hit DVE 2x/4x modes):
  Pool: P = d*t + c          (tensor_scalar, dtype-blind engine)
  DVE : P *= t               (tensor_tensor, 2x)
  Act : P += b               (Identity + per-partition bias)
  DVE : P *= t               (2x)
  Act : P += a  -> M fp16
  PE  : 8x transpose (fp16 identity, 1 cyc/row)
  DVE : wc = psum + W^T      (fused PSUM evacuation + weight add, 2x)
  PE  : 8x matmul, rhs = x fp8 (1 cyc/row), accumulate f32 PSUM

Sharding: hidden dim split across 8 cores (h_loc = 256); x replicated.
Host does layout/dtype prep only (transposes, index extraction, sharding).
"""

import numpy as np
import ml_dtypes

B, I, H, NB = 512, 4096, 2048, 4
NCORES = 8
H_LOC = H // NCORES          # 256
N_HT = H_LOC // 128          # 2 partition tiles of hidden per core
S = 1024                     # i-supertile size
N_SUPER = I // S             # 4 supers per h-tile
NJ = S // 128                # 8 k-chunks per supertile
N_CHUNK = I // 128           # 32 matmul k-chunks

TRACE = False
LAST_RESULTS = None
_CACHED = {}


def _build_bass(reps=1):
    import concourse.bacc as bacc
    import concourse.mybir as mybir
    from concourse.tile import TileContext
    from concourse.masks import make_identity

    f32 = mybir.dt.float32
    fp16 = mybir.dt.float16
    fp8 = mybir.dt.float8e4
    AF = mybir.ActivationFunctionType
    ALU = mybir.AluOpType

    nc = bacc.Bacc("TRN2", target_bir_lowering=False, debug=False)

    xT = nc.dram_tensor("xT", [I, B], fp8, kind="ExternalInput")
    wT_in = nc.dram_tensor("wT", [I, H_LOC], fp16, kind="ExternalInput")
    idx_in = nc.dram_tensor("idx", [H_LOC, I], fp16, kind="ExternalInput")
    tau_n = nc.dram_tensor("tau_n", [H_LOC, NB], f32, kind="ExternalInput")
    tau_m = nc.dram_tensor("tau_m", [H_LOC, 1], f32, kind="ExternalInput")
    b_in = nc.dram_tensor("b", [H_LOC, 1], f32, kind="ExternalInput")
    out = nc.dram_tensor("out", [H_LOC, B], fp8, kind="ExternalOutput")

    # dram views grouped by supertile: [super, p, chunk-in-super, free]
    xT_v = xT.rearrange("(g c p) b -> g p c b", p=128, c=NJ)
    wT_v = wT_in.rearrange("(g c p) h -> g p c h", p=128, c=NJ)

    with TileContext(nc) as tc:
        with (
            tc.tile_pool(name="const", bufs=1) as const_pool,
            tc.tile_pool(name="xp", bufs=1) as x_pool,
            tc.tile_pool(name="wt", bufs=1) as wt_pool,
            tc.tile_pool(name="ix", bufs=N_HT * N_SUPER) as idx_pool,
            tc.tile_pool(name="mp", bufs=4) as m_pool,
            tc.tile_pool(name="wc", bufs=4) as wc_pool,
            tc.tile_pool(name="res", bufs=2) as res_pool,
            tc.tile_pool(name="pt", bufs=3, space="PSUM") as psum_t_pool,
            tc.tile_pool(name="po", bufs=2, space="PSUM") as psum_o_pool,
            tc.tile_pool(name="pw", bufs=1, space="PSUM") as psum_w_pool,
        ):
            ident = const_pool.tile([128, 128], fp16)
            make_identity(nc, ident)

            # HAM warmup: the PE idles while the first M plane is built and
            # its clock would otherwise sit at 0.65/1.2 GHz when real work
            # arrives.  Spin dummy matmuls from t=0 so the array is at
            # 2.4 GHz (>3us continuously busy) when the transposes start.
            warm = psum_w_pool.tile([128, 128], f32, name="warm")
            for wi in range(26):
                nc.tensor.matmul(warm[:], ident[:], ident[:],
                                 start=True, stop=True, skip_group_check=True)
            for rep in range(reps):
                _emit_rep(nc, tc, rep, ident,
                          const_pool, x_pool, wt_pool, idx_pool, m_pool,
                          wc_pool, res_pool, psum_t_pool, psum_o_pool,
                          xT_v, wT_v, idx_in, tau_n, tau_m, b_in, out,
                          f32, fp16, fp8, AF, ALU)

    nc.compile()
    return nc


def _emit_rep(nc, tc, rep, ident,
              const_pool, x_pool, wt_pool, idx_pool, m_pool,
              wc_pool, res_pool, psum_t_pool, psum_o_pool,
              xT_v, wT_v, idx_in, tau_n, tau_m, b_in, out,
              f32, fp16, fp8, AF, ALU):
    R = f"r{rep}_"

    # tiny parameter DMAs first so the coefficient math can start at once
    param_tiles = []
    for ht in range(N_HT):
        hs = slice(ht * 128, (ht + 1) * 128)
        tn = const_pool.tile([128, NB], f32, tag=f"{R}tn{ht}", name=f"{R}tn{ht}")
        tm = const_pool.tile([128, 1], f32, tag=f"{R}tm{ht}", name=f"{R}tm{ht}")
        bv = const_pool.tile([128, 1], f32, tag=f"{R}bv{ht}", name=f"{R}bv{ht}")
        nc.sync.dma_start(tn[:], tau_n[hs, :])
        nc.sync.dma_start(tm[:], tau_m[hs, :])
        nc.sync.dma_start(bv[:], b_in[hs, :])
        param_tiles.append((tn, tm, bv))

    # bulk DMAs, interleaved so each supertile's inputs land in work order
    x_sb = x_pool.tile([128, N_CHUNK, B], fp8, tag="xsb", name=f"{R}x")
    wt_sb = wt_pool.tile([128, N_CHUNK, H_LOC], fp16, tag="wtsb", name=f"{R}wt")
    idx_sb = {}
    for ig in range(N_SUPER):
        cs = slice(ig * NJ, (ig + 1) * NJ)
        isl = slice(ig * S, (ig + 1) * S)
        for ht in range(N_HT):
            hs = slice(ht * 128, (ht + 1) * 128)
            it = idx_pool.tile([128, S], fp16, tag="ix", name=f"{R}ix{ht}_{ig}")
            nc.sync.dma_start(it[:], idx_in[hs, isl])
            idx_sb[(ht, ig)] = it
        nc.sync.dma_start(wt_sb[:, cs, :], wT_v[ig])
        nc.sync.dma_start(x_sb[:, cs, :], xT_v[ig])

    # per-h-tile parameters: monomial coefs of the cubic through
    # (k, oma[:,k]), k=0..3, and the spike threshold.  Both sigmoids are
    # emitted before any Identity op so the Act table only loads twice.
    sig_t = []
    for ht in range(N_HT):
        tn, tm, bv = param_tiles[ht]
        sig_n = const_pool.tile([128, NB], f32, tag=f"{R}sn{ht}", name=f"{R}sn{ht}")
        nc.scalar.activation(sig_n[:], tn[:], AF.Sigmoid)
        beta = const_pool.tile([128, 1], f32, tag=f"{R}be{ht}", name=f"{R}be{ht}")
        nc.scalar.activation(beta[:], tm[:], AF.Sigmoid)
        sig_t.append((sig_n, beta))

    coef = []   # (a_ap, b_ap, c_ap, d_ap) per ht
    thr_t = []
    for ht in range(N_HT):
        tn, tm, bv = param_tiles[ht]
        sig_n, beta = sig_t[ht]
        oma = const_pool.tile([128, NB], f32, tag=f"{R}oma{ht}", name=f"{R}oma{ht}")
        # 0.5 * (1 - sigmoid(tau_n)) — includes the 0.5 dendritic scale
        nc.vector.tensor_scalar(oma[:], sig_n[:], -0.5, 0.5, op0=ALU.mult, op1=ALU.add)

        # Newton -> monomial coefficients: p(t) = a + b t + c t^2 + d t^3
        sc = const_pool.tile([128, 12], f32, tag=f"{R}sc{ht}", name=f"{R}sc{ht}")
        o = lambda k: oma[:, k : k + 1]
        d3 = sc[:, 0:3]                       # first differences
        dd2 = sc[:, 3:5]                      # second differences
        ddd = sc[:, 5:6]                      # third difference
        b_c, c_c, d_c = sc[:, 6:7], sc[:, 7:8], sc[:, 8:9]
        t1, t2 = sc[:, 9:10], sc[:, 10:11]
        nc.vector.tensor_tensor(d3, oma[:, 1:4], oma[:, 0:3], ALU.subtract)
        nc.vector.tensor_tensor(dd2, d3[:, 1:3], d3[:, 0:2], ALU.subtract)
        nc.vector.tensor_tensor(ddd, dd2[:, 1:2], dd2[:, 0:1], ALU.subtract)
        d0, dd0 = d3[:, 0:1], dd2[:, 0:1]
        nc.vector.tensor_scalar(d_c, ddd, 1.0 / 6.0, None, op0=ALU.mult)
        nc.vector.tensor_tensor(c_c, dd0, ddd, ALU.subtract)
        nc.vector.tensor_scalar(c_c, c_c, 0.5, None, op0=ALU.mult)
        # b = d0 - dd0/2 + ddd/3
        nc.vector.tensor_scalar(t2, ddd, 1.0 / 3.0, None, op0=ALU.mult)
        nc.vector.tensor_scalar(t1, dd0, -0.5, None, op0=ALU.mult)
        nc.vector.tensor_tensor(t1, t1, d0, ALU.add)
        nc.vector.tensor_tensor(b_c, t1, t2, ALU.add)
        coef.append((o(0), b_c, c_c, d_c))

        omb = const_pool.tile([128, 1], f32, tag=f"{R}ob{ht}", name=f"{R}ob{ht}")
        nc.vector.tensor_scalar(omb[:], beta[:], -1.0, 1.0, op0=ALU.mult, op1=ALU.add)
        rb = const_pool.tile([128, 1], f32, tag=f"{R}rb{ht}", name=f"{R}rb{ht}")
        nc.vector.reciprocal(rb[:], omb[:])
        thr = const_pool.tile([128, 1], f32, tag=f"{R}th{ht}", name=f"{R}th{ht}")
        nc.vector.tensor_tensor(thr[:], rb[:], bv[:], ALU.subtract)
        thr_t.append(thr)

    # main software pipeline over supertiles, alternating h-tiles
    supers = [(ht, ig) for ig in range(N_SUPER) for ht in range(N_HT)]
    n_sup = len(supers)
    psum_out = [None] * N_HT
    for ht in range(N_HT):
        psum_out[ht] = psum_o_pool.tile([128, B], f32, tag="po", name=f"{R}po{ht}")

    P_t = [None] * n_sup
    pt_t = [None] * n_sup
    wc_t = [None] * n_sup

    def sA(k):  # Pool: P = d*t + c
        ht, ig = supers[k]
        _, _, c_ap, d_ap = coef[ht]
        P = m_pool.tile([128, S], fp16, tag="mp", name=f"{R}P{k}")
        P_t[k] = P
        nc.gpsimd.tensor_scalar(P[:], idx_sb[(ht, ig)][:], d_ap, c_ap,
                                op0=ALU.mult, op1=ALU.add)

    def sB(k):  # DVE: P *= t
        ht, ig = supers[k]
        nc.vector.tensor_tensor(P_t[k][:], P_t[k][:], idx_sb[(ht, ig)][:], ALU.mult)

    def sC(k):  # Act: P += b
        ht, ig = supers[k]
        nc.scalar.activation(P_t[k][:], P_t[k][:], AF.Identity, bias=coef[ht][1])

    def sD(k):  # DVE: P *= t
        ht, ig = supers[k]
        nc.vector.tensor_tensor(P_t[k][:], P_t[k][:], idx_sb[(ht, ig)][:], ALU.mult)

    def sE(k):  # Act: P += a  -> final fp16 M plane
        ht, ig = supers[k]
        nc.scalar.activation(P_t[k][:], P_t[k][:], AF.Identity, bias=coef[ht][0])

    def sF(k):  # PE: 8 transposes into fp16 PSUM
        pt = psum_t_pool.tile([128, NJ, 128], fp16, tag="pt", name=f"{R}pt{k}")
        pt_t[k] = pt
        P = P_t[k]
        for j in range(NJ):
            nc.tensor.transpose(pt[:, j, :], P[:, j * 128 : (j + 1) * 128], ident[:])

    def sG(k):  # DVE: wc = pt + W^T   (PSUM evacuation fused with +W)
        ht, ig = supers[k]
        wc = wc_pool.tile([128, NJ, 128], fp16, tag="wc", name=f"{R}wc{k}")
        wc_t[k] = wc
        cs = slice(ig * NJ, (ig + 1) * NJ)
        hsl = slice(ht * 128, (ht + 1) * 128)
        nc.vector.tensor_tensor(wc[:], pt_t[k][:], wt_sb[:, cs, hsl], ALU.add)

    def sH(k):  # PE: 8 matmuls, rhs = x fp8
        ht, ig = supers[k]
        po = psum_out[ht]
        wc = wc_t[k]
        for j in range(NJ):
            c = ig * NJ + j
            nc.tensor.matmul(
                po[:], wc[:, j, :], x_sb[:, c, :],
                start=(c == 0), stop=(c == N_CHUNK - 1),
                skip_group_check=True,
            )
        wc_t[k] = None

    # pipelined emission: stage X of super k runs alongside stage X+1 of
    # super k-1; PE sees tr(k+1) before mm(k) so the wc add has slack.
    for k in range(n_sup + 4):
        if k < n_sup:
            sA(k)
        if 0 <= k - 1 < n_sup:
            sB(k - 1)
            sC(k - 1)
        if 0 <= k - 2 < n_sup:
            sD(k - 2)
            sE(k - 2)
        if 0 <= k - 3 < n_sup:
            sF(k - 3)
            sG(k - 3)
        if 0 <= k - 4 < n_sup:
            sH(k - 4)

    for ht in range(N_HT):
        res = res_pool.tile([128, B], fp8, tag="res", name=f"{R}res{ht}")
        nc.vector.tensor_scalar(
            res[:], psum_out[ht][:], thr_t[ht][:], None, op0=ALU.is_ge
        )
        nc.sync.dma_start(out[ht * 128 : (ht + 1) * 128, :], res[:])


def _get_nc(reps=1):
    key = f"nc{reps}"
    if key not in _CACHED:
        _CACHED[key] = _build_bass(reps)
    return _CACHED[key]


def kernel(**inputs):
    global LAST_RESULTS
    from concourse.bass_utils import run_bass_kernel_spmd

    x = np.asarray(inputs["x"], dtype=np.float32)
    W = np.asarray(inputs["W"], dtype=np.float32)
    b = np.asarray(inputs["b"], dtype=np.float32)
    tau_m = np.asarray(inputs["tau_m"], dtype=np.float32)
    tau_n = np.asarray(inputs["tau_n"], dtype=np.float32)
    mask = np.asarray(inputs["mask"], dtype=np.float32)

    fp16 = np.float16
    fp8 = ml_dtypes.float8_e4m3
    xT = np.ascontiguousarray(x.T).astype(fp8)                        # [I, B]
    # branch index of each (h, i): mask is one-hot over k (exact 0/1 values)
    idx = (
        mask[:, :, 1] + 2.0 * mask[:, :, 2] + 3.0 * mask[:, :, 3]
    ).astype(fp16)                                                    # [H, I]
    W16 = W.astype(fp16)

    nc = _get_nc()
    in_maps = []
    for c in range(NCORES):
        hs = slice(c * H_LOC, (c + 1) * H_LOC)
        in_maps.append({
            "xT": xT,
            "wT": np.ascontiguousarray(W16[hs].T),
            "idx": np.ascontiguousarray(idx[hs]),
            "tau_n": np.ascontiguousarray(tau_n[hs]),
            "tau_m": np.ascontiguousarray(tau_m[hs, None]),
            "b": np.ascontiguousarray(b[hs, None]),
        })

    try:
        res = run_bass_kernel_spmd(
            nc, in_maps, core_ids=list(range(NCORES)), trace=TRACE,
        )
    except Exception:
        if not TRACE:
            raise
        # tracing needs the NTFF profiling hook, which not every
        # environment provides — rerun without it
        res = run_bass_kernel_spmd(
            nc, in_maps, core_ids=list(range(NCORES)), trace=False,
        )
    LAST_RESULTS = res
    outT = np.concatenate(
        [np.asarray(r["out"], dtype=np.float32) for r in res.results], axis=0
    )                                                                 # [H, B]
    return np.ascontiguousarray(outT.T)                               # [B, H]
